# revision 7
# baseline (speedup 1.0000x reference)
"""Trainium2 Bass kernel for the CSMAdapter module.

Contract: kernel(**inputs) takes the FULL unsharded inputs (as produced by
the reference setup_inputs()) and returns the FULL output [4, 100, 1024].

Strategy
--------
All weight-only computation is folded on the host (it is data-independent):
    w_proj   = W_in @ Wd.T + bd
    w_prime  = P.T @ w_proj @ P
    masked_w = w_prime * sigmoid(spectral_mask)
    A        = P @ masked_w.T @ P.T          # fused = x @ A
    W_big    = W_in.T @ A                    # fused = llama @ W_big + b_in @ A
The final LayerNorm + mel projection algebra is folded into the mel GEMM:
    mel[m,t] = rstd[t]*(Wg @ h2)[m,t] - (mu[t]*rstd[t])*c1[m] + c2[m]
with Wg = Wmel * ln_g, c1 = Wmel @ ln_g, c2 = Wmel @ ln_b + bmel.

Device (SPMD over 8 cores, data-parallel over the 4096 tokens, 512 each +
2-token conv halos):
    fused_ext = llama_ext @ W_big + b_big (x) vmask     (one K=3072 GEMM)
    conv1 -> gelu -> conv2 as block-diagonal per-tap matmuls (groups=16)
    channel mean / mean-square via ones-vector matmuls
    mel GEMM + rank-1 correction matmuls
Matmuls run in float32r (full fp32 data, fast PE mode).
"""

import sys

import numpy as np


def _ensure_concourse():
    try:
        import concourse  # noqa: F401
    except ImportError:  # pragma: no cover
        for p in ("/opt/trn_rl_repo", "/root/.axon_site/_ro/trn_rl_repo"):
            if p not in sys.path:
                sys.path.insert(0, p)


# ---- static shapes ----
B, T, L, D = 4, 1024, 3072, 1024
NCORES = 8
TOK = 512            # owned tokens per core
EXT = TOK + 4        # fused ext window: tokens -2 .. TOK+2
G1E = TOK + 2        # conv1 ext output: tokens -1 .. TOK+1
KT = L // 128        # 24
DT = D // 128        # 8
NMEL = 100
HB = EXT // 2        # 258  big-GEMM halves
H1 = G1E // 2        # 257  conv1 halves
H2 = TOK // 2        # 256  conv2 halves
GS = 64              # group size (1024 / 16 groups)

OFF_BB = 0
OFF_VM = OFF_BB + D
OFF_C1 = OFF_VM + EXT
OFF_C2 = OFF_C1 + NMEL
OFF_EPS = OFF_C2 + NMEL
OFF_ONES = OFF_EPS + 1
SM_LEN = OFF_ONES + TOK

LN_EPS = 1e-5

_PROGRAM = None          # cached (nc, input names)
LAST_RESULTS = None      # BassKernelResults of the most recent run (for test.py)


def _build_program():
    _ensure_concourse()
    from concourse import bacc, tile
    import concourse.mybir as mybir

    f32 = mybir.dt.float32
    f32r = mybir.dt.float32r
    AF = mybir.ActivationFunctionType

    nc = bacc.Bacc("TRN2", debug=False, target_bir_lowering=False)

    xt_d = nc.dram_tensor("xt", [KT, 128, EXT], f32r, kind="ExternalInput")
    wbig_d = nc.dram_tensor("wbig", [DT, KT, 128, 128], f32r, kind="ExternalInput")
    cw1_d = nc.dram_tensor("cw1", [DT, 3, 128, 128], f32r, kind="ExternalInput")
    cw2_d = nc.dram_tensor("cw2", [DT, 3, 128, 128], f32r, kind="ExternalInput")
    wgt_d = nc.dram_tensor("wgt", [DT, 128, NMEL], f32r, kind="ExternalInput")
    cb_d = nc.dram_tensor("cb", [128, 27], f32, kind="ExternalInput")
    sm_d = nc.dram_tensor("smalls", [1, SM_LEN], f32r, kind="ExternalInput")
    onec_d = nc.dram_tensor("onec", [128, 1], f32r, kind="ExternalInput")
    mel_d = nc.dram_tensor("mel", [NMEL, TOK], f32, kind="ExternalOutput")

    with tile.TileContext(nc) as tc:
        with (
            tc.tile_pool(name="consts", bufs=1) as consts,
            tc.tile_pool(name="wpool", bufs=2) as wpool,
            tc.tile_pool(name="acts", bufs=1) as acts,
            tc.tile_pool(name="stats", bufs=1) as stats,
            tc.tile_pool(name="ps_mm", bufs=3, space="PSUM") as ps_mm,
            tc.tile_pool(name="ps_st", bufs=2, space="PSUM") as ps_st,
            tc.tile_pool(name="ps_mel", bufs=3, space="PSUM") as ps_mel,
        ):
            # ---- constants / small tensors ----
            sm_sb = consts.tile([1, SM_LEN], f32r, name="sm_sb")
            nc.sync.dma_start(out=sm_sb, in_=sm_d[:])
            cb_sb = consts.tile([128, 27], f32, name="cb_sb")
            nc.sync.dma_start(out=cb_sb, in_=cb_d[:])
            ones_col = consts.tile([128, 1], f32r, name="ones_col")
            nc.sync.dma_start(out=ones_col, in_=onec_d[:])
            ones_row = sm_sb[0:1, OFF_ONES : OFF_ONES + TOK]

            # ---- first W slice, then input tokens, then conv/mel weights ----
            wb_tiles = []
            wb0 = wpool.tile([128, KT, 128], f32r, name="wb0", tag="wb")
            nc.sync.dma_start(out=wb0, in_=wbig_d[0].rearrange("k p c -> p k c"))
            wb_tiles.append(wb0)

            xk = []
            for k in range(KT):
                t = consts.tile([128, EXT], f32r, name=f"xk{k}", tag=f"xk{k}")
                nc.sync.dma_start(out=t, in_=xt_d[k])
                xk.append(t)

            cw1_sb = consts.tile([128, DT, 3, 128], f32r, name="cw1_sb")
            nc.sync.dma_start(out=cw1_sb, in_=cw1_d.rearrange("d t p c -> p d t c"))
            cw2_sb = consts.tile([128, DT, 3, 128], f32r, name="cw2_sb")
            nc.sync.dma_start(out=cw2_sb, in_=cw2_d.rearrange("d t p c -> p d t c"))
            wgt_sb = consts.tile([128, DT, NMEL], f32r, name="wgt_sb")
            nc.sync.dma_start(out=wgt_sb, in_=wgt_d.rearrange("d p m -> p d m"))

            # ---- phase A: fused_ext[d] = llama_ext @ W_big + b_big (x) vmask ----
            fused = []
            for d in range(DT):
                if d + 1 < DT:
                    wbn = wpool.tile([128, KT, 128], f32r, name=f"wb{d + 1}", tag="wb")
                    nc.sync.dma_start(
                        out=wbn, in_=wbig_d[d + 1].rearrange("k p c -> p k c")
                    )
                    wb_tiles.append(wbn)
                fu = acts.tile([128, EXT], f32r, name=f"fu{d}", tag=f"fu{d}")
                fused.append(fu)
                wb = wb_tiles[d]
                for h in range(2):
                    ps = ps_mm.tile([128, HB], f32, name=f"psA{d}_{h}", tag="mm")
                    sl = slice(h * HB, (h + 1) * HB)
                    for k in range(KT):
                        nc.tensor.matmul(
                            ps,
                            lhsT=wb[:, k, :],
                            rhs=xk[k][:, sl],
                            start=(k == 0),
                            stop=False,
                        )
                    nc.tensor.matmul(
                        ps,
                        lhsT=sm_sb[0:1, OFF_BB + d * 128 : OFF_BB + (d + 1) * 128]
                        ,
                        rhs=sm_sb[0:1, OFF_VM + h * HB : OFF_VM + (h + 1) * HB]
                        ,
                        start=False,
                        stop=True,
                    )
                    nc.scalar.copy(out=fu[:, sl], in_=ps)

            # ---- phase B: conv1 + exact gelu (via Erf) -> g ----
            # fp32r matmuls need an even moving free dim: halves of 258 + 256.
            B_STARTS = (0, 258)
            B_WIDTHS = (258, 256)
            g = []
            for d in range(DT):
                gd = acts.tile([128, G1E], f32r, name=f"g{d}", tag=f"g{d}")
                g.append(gd)
                for h in range(2):
                    st, w = B_STARTS[h], B_WIDTHS[h]
                    ps = ps_mm.tile([128, w], f32, name=f"psB{d}_{h}", tag="mm")
                    for tap in range(3):
                        nc.tensor.matmul(
                            ps,
                            lhsT=cw1_sb[:, d, tap, :],
                            rhs=fused[d][:, st + tap : st + tap + w],
                            start=(tap == 0),
                            stop=(tap == 2),
                        )
                    # e = erf((conv1 + b1) / sqrt(2))  [bias column pre-scaled]
                    e = acts.tile([128, 258], f32, name=f"e{d}_{h}", tag="e",
                                  bufs=3)
                    nc.scalar.activation(
                        out=e[:, :w],
                        in_=ps,
                        func=AF.Erf,
                        bias=cb_sb[:, d : d + 1],
                        scale=0.7071067811865476,
                    )
                    # h1b = conv1 + b1 (raw bias)
                    h1b = acts.tile([128, 258], f32, name=f"h1b{d}_{h}",
                                    tag="h1b", bufs=3)
                    nc.vector.tensor_scalar_add(
                        h1b[:, :w], ps, cb_sb[:, 19 + d : 20 + d]
                    )
                    # t = 0.5*e + 0.5 ; g = t * h1b
                    tt = acts.tile([128, 258], f32, name=f"t{d}_{h}", tag="tt",
                                   bufs=3)
                    nc.vector.tensor_scalar(
                        tt[:, :w], e[:, :w], 0.5, 0.5,
                        op0=mybir.AluOpType.mult, op1=mybir.AluOpType.add,
                    )
                    nc.vector.tensor_mul(gd[:, st : st + w], tt[:, :w],
                                         h1b[:, :w])
                # zero the conv2 halo columns at sequence edges (data-driven)
                nc.vector.tensor_mul(gd[:, 0:1], gd[:, 0:1], cb_sb[:, 16:17])
                nc.vector.tensor_mul(
                    gd[:, G1E - 1 : G1E], gd[:, G1E - 1 : G1E], cb_sb[:, 17:18]
                )

            # ---- phase C: conv2 -> h2, h2sq ----
            h2 = []
            h2sq = []
            for d in range(DT):
                h2d = acts.tile([128, TOK], f32r, name=f"h2{d}", tag=f"h2{d}")
                h2sqd = acts.tile([128, TOK], f32r, name=f"h2sq{d}", tag=f"h2sq{d}")
                h2.append(h2d)
                h2sq.append(h2sqd)
                for h in range(2):
                    ps = ps_mm.tile([128, H2], f32, name=f"psC{d}_{h}", tag="mm")
                    for tap in range(3):
                        nc.tensor.matmul(
                            ps,
                            lhsT=cw2_sb[:, d, tap, :],
                            rhs=g[d][:, h * H2 + tap : h * H2 + tap + H2]
                            ,
                            start=(tap == 0),
                            stop=(tap == 2),
                        )
                    sl = slice(h * H2, (h + 1) * H2)
                    nc.scalar.add(out=h2d[:, sl], in_=ps, add=cb_sb[:, 8 + d : 9 + d])
                    nc.scalar.activation(
                        out=h2sqd[:, sl],
                        in_=ps,
                        func=AF.Square,
                        bias=cb_sb[:, 8 + d : 9 + d],
                        scale=1.0,
                    )

            # ---- phase D: channel sums via ones-matmuls ----
            ps_mu = ps_st.tile([1, TOK], f32, name="ps_mu", tag="st")
            for d in range(DT):
                nc.tensor.matmul(
                    ps_mu,
                    lhsT=ones_col,
                    rhs=h2[d][:],
                    start=(d == 0),
                    stop=(d == DT - 1),
                )
            ps_sq = ps_st.tile([1, TOK], f32, name="ps_sq", tag="st")
            for d in range(DT):
                nc.tensor.matmul(
                    ps_sq,
                    lhsT=ones_col,
                    rhs=h2sq[d][:],
                    start=(d == 0),
                    stop=(d == DT - 1),
                )

            # ---- phase E: stats on [1, TOK] ----
            mean = stats.tile([1, TOK], f32, name="mean")
            nc.vector.tensor_scalar_mul(mean, ps_mu, 1.0 / D)
            msq = stats.tile([1, TOK], f32, name="msq")
            nc.scalar.activation(msq, mean, AF.Square)
            varr = stats.tile([1, TOK], f32, name="varr")
            nc.vector.tensor_scalar_mul(varr, ps_sq, 1.0 / D)
            var = stats.tile([1, TOK], f32, name="var")
            nc.vector.tensor_sub(var, varr, msq)
            sqv = stats.tile([1, TOK], f32, name="sqv")
            nc.scalar.activation(
                sqv, var, AF.Sqrt, bias=cb_sb[0:1, 18:19], scale=1.0
            )
            rstd32 = stats.tile([1, TOK], f32, name="rstd32")
            nc.vector.reciprocal(rstd32, sqv)
            rstd = stats.tile([1, TOK], f32r, name="rstd")
            nc.vector.tensor_copy(rstd, rstd32)
            negu = stats.tile([1, TOK], f32r, name="negu")
            nc.vector.scalar_tensor_tensor(
                negu,
                in0=mean,
                scalar=-1.0,
                in1=rstd32,
                op0=mybir.AluOpType.mult,
                op1=mybir.AluOpType.mult,
            )

            # ---- phase F: mel GEMM + rank-1 corrections ----
            ps_m = ps_mel.tile([NMEL, TOK], f32, name="ps_m", tag="mel")
            for d in range(DT):
                nc.tensor.matmul(
                    ps_m,
                    lhsT=wgt_sb[:, d, :],
                    rhs=h2[d][:],
                    start=(d == 0),
                    stop=(d == DT - 1),
                )
            ps_r = ps_mel.tile([NMEL, TOK], f32, name="ps_r", tag="mel")
            nc.tensor.matmul(
                ps_r,
                lhsT=sm_sb[0:1, OFF_C1 : OFF_C1 + NMEL],
                rhs=negu[0:1, :],
                start=True,
                stop=False,
            )
            nc.tensor.matmul(
                ps_r,
                lhsT=sm_sb[0:1, OFF_C2 : OFF_C2 + NMEL],
                rhs=ones_row,
                start=False,
                stop=True,
            )
            ps_s = ps_mel.tile([NMEL, TOK], f32, name="ps_s", tag="mel")
            nc.tensor.matmul(
                ps_s,
                lhsT=sm_sb[0:1, OFF_ONES : OFF_ONES + NMEL],
                rhs=rstd[0:1, :],
                start=True,
                stop=True,
            )
            s_sb = stats.tile([NMEL, TOK], f32, name="s_sb")
            nc.vector.tensor_copy(s_sb, ps_s)
            t1 = stats.tile([NMEL, TOK], f32, name="t1")
            nc.vector.tensor_mul(t1, ps_m, s_sb)
            out_sb = stats.tile([NMEL, TOK], f32, name="out_sb")
            nc.vector.tensor_add(out_sb, t1, ps_r)
            nc.sync.dma_start(out=mel_d[:], in_=out_sb)

    nc.compile()
    return nc


def _sigmoid64(x):
    return 1.0 / (1.0 + np.exp(-x.astype(np.float64)))


def host_prep(inputs):
    """Fold all data-independent computation; build per-core device inputs.

    Returns (shared, per_core) where shared is a dict of replicated arrays
    and per_core is a list of 8 dicts with the core-specific arrays.
    """
    f32 = np.float32
    W_in = np.asarray(inputs["W_in"], dtype=np.float64)
    Wd = np.asarray(inputs["Wd"], dtype=np.float64)
    bd = np.asarray(inputs["bd"], dtype=np.float64)
    P = np.asarray(inputs["P"], dtype=np.float64)
    smask = np.asarray(inputs["spectral_mask"], dtype=np.float64)
    b_in = np.asarray(inputs["b_in"], dtype=np.float64)

    w_proj = W_in @ Wd.T + bd[None, :]
    w_prime = P.T @ w_proj @ P
    masked_w = w_prime * _sigmoid64(smask)
    A = P @ masked_w.T @ P.T
    W_big = np.ascontiguousarray((W_in.T @ A), dtype=f32)      # [L, D]
    b_big = (b_in @ A).astype(f32)                             # [D]

    # [d, k, kp, dc]
    wbig_t = np.ascontiguousarray(
        W_big.reshape(KT, 128, DT, 128).transpose(2, 0, 1, 3)
    )

    def blockdiag(w):
        w = np.asarray(w, dtype=f32)  # [C, GS, 3]
        out = np.zeros((DT, 3, 128, 128), dtype=f32)
        for d in range(DT):
            for co in range(128):
                c = d * 128 + co
                blk = co // GS
                # out[d, tap, blk*GS + i, co] = w[c, i, tap]
                out[d, :, blk * GS : (blk + 1) * GS, co] = w[c].T
        return out

    cw1_t = blockdiag(inputs["conv1_w"])
    cw2_t = blockdiag(inputs["conv2_w"])

    Wmel = np.asarray(inputs["Wmel"], dtype=np.float64)
    ln_g = np.asarray(inputs["ln_g"], dtype=np.float64)
    ln_b = np.asarray(inputs["ln_b"], dtype=np.float64)
    bmel = np.asarray(inputs["bmel"], dtype=np.float64)
    Wg = (Wmel * ln_g[None, :]).astype(f32)                    # [NMEL, D]
    wgt_t = np.ascontiguousarray(Wg.T.reshape(DT, 128, NMEL))  # [d, kp, m]
    c1 = (Wmel @ ln_g).astype(f32)
    c2 = (Wmel @ ln_b + bmel).astype(f32)

    cb_base = np.zeros((128, 27), dtype=f32)
    cb_base[:, 18] = LN_EPS
    b1_cols = np.asarray(inputs["conv1_b"], dtype=f32).reshape(DT, 128).T
    cb_base[:, 0:8] = b1_cols * np.float32(0.7071067811865476)  # pre-scaled for Erf
    cb_base[:, 8:16] = np.asarray(inputs["conv2_b"], dtype=f32).reshape(DT, 128).T
    cb_base[:, 19:27] = b1_cols

    llama = np.asarray(inputs["llama_embeddings"], dtype=f32).reshape(B * T, L)

    shared = dict(wbig=wbig_t, cw1=cw1_t, cw2=cw2_t, wgt=wgt_t,
                  onec=np.ones((128, 1), dtype=f32))
    per_core = []
    for c in range(NCORES):
        b, h = divmod(c, 2)
        start = b * T + h * TOK
        ext_idx = np.arange(start - 2, start + TOK + 2)
        valid = (ext_idx >= b * T) & (ext_idx < (b + 1) * T)
        xext = np.zeros((EXT, L), dtype=f32)
        xext[valid] = llama[ext_idx[valid]]
        xt = np.ascontiguousarray(xext.T).reshape(KT, 128, EXT)

        sm = np.zeros((1, SM_LEN), dtype=f32)
        sm[0, OFF_BB : OFF_BB + D] = b_big
        sm[0, OFF_VM : OFF_VM + EXT] = valid.astype(f32)
        sm[0, OFF_C1 : OFF_C1 + NMEL] = c1
        sm[0, OFF_C2 : OFF_C2 + NMEL] = c2
        sm[0, OFF_EPS] = LN_EPS
        sm[0, OFF_ONES : OFF_ONES + TOK] = 1.0

        cb = cb_base.copy()
        # g halo validity: col 16 -> token -1, col 17 -> token TOK
        cb[:, 16] = 1.0 if h == 1 else 0.0
        cb[:, 17] = 1.0 if h == 0 else 0.0

        per_core.append(dict(xt=xt, smalls=sm, cb=cb))
    return shared, per_core


def kernel(**inputs):
    global _PROGRAM, LAST_RESULTS
    _ensure_concourse()
    from concourse import bass_utils

    if _PROGRAM is None:
        _PROGRAM = _build_program()
    nc = _PROGRAM

    shared, per_core = host_prep(inputs)
    in_maps = [{**shared, **pc} for pc in per_core]

    res = bass_utils.run_bass_kernel_spmd(nc, in_maps, core_ids=list(range(NCORES)))
    LAST_RESULTS = res

    out = np.zeros((B, NMEL, T), dtype=np.float32)
    for c in range(NCORES):
        b, h = divmod(c, 2)
        out[b, :, h * TOK : (h + 1) * TOK] = res.results[c]["mel"]
    return out


# revision 9
# speedup vs baseline: 1.2552x; 1.2552x over previous
"""Trainium2 Bass kernel for the CSMAdapter module.

Contract: kernel(**inputs) takes the FULL unsharded inputs (as produced by
the reference setup_inputs()) and returns the FULL output [4, 100, 1024].

Strategy
--------
All weight-only computation is folded on the host (it is data-independent):
    w_proj   = W_in @ Wd.T + bd
    w_prime  = P.T @ w_proj @ P
    masked_w = w_prime * sigmoid(spectral_mask)
    A        = P @ masked_w.T @ P.T          # fused = x @ A
    W_big    = W_in.T @ A                    # fused = llama @ W_big + b_in @ A
The final LayerNorm + mel projection algebra is folded into the mel GEMM:
    mel[m,t] = rstd[t]*(Wg @ h2)[m,t] - (mu[t]*rstd[t])*c1[m] + c2[m]
with Wg = Wmel * ln_g, c1 = Wmel @ ln_g, c2 = Wmel @ ln_b + bmel.

Device (SPMD over 8 cores, data-parallel over the 4096 tokens, 512 each +
2-token conv halos):
    fused_ext = llama_ext @ W_big + b_big (x) vmask     (one K=3072 GEMM)
    conv1 -> gelu -> conv2 as block-diagonal per-tap matmuls (groups=16)
    channel mean / mean-square via ones-vector matmuls
    mel GEMM + rank-1 correction matmuls
Matmuls run in float32r (full fp32 data, fast PE mode).
"""

import sys

import numpy as np


def _ensure_concourse():
    try:
        import concourse  # noqa: F401
    except ImportError:  # pragma: no cover
        for p in ("/opt/trn_rl_repo", "/root/.axon_site/_ro/trn_rl_repo"):
            if p not in sys.path:
                sys.path.insert(0, p)


# ---- static shapes ----
B, T, L, D = 4, 1024, 3072, 1024
NCORES = 8
TOK = 512            # owned tokens per core
EXT = TOK + 4        # fused ext window: tokens -2 .. TOK+2
G1E = TOK + 2        # conv1 ext output: tokens -1 .. TOK+1
KT = L // 128        # 24
DT = D // 128        # 8
NMEL = 100
HB = EXT // 2        # 258  big-GEMM halves
H1 = G1E // 2        # 257  conv1 halves
H2 = TOK // 2        # 256  conv2 halves
GS = 64              # group size (1024 / 16 groups)

OFF_BB = 0
OFF_VM = OFF_BB + D
OFF_C1 = OFF_VM + EXT
OFF_C2 = OFF_C1 + NMEL
OFF_EPS = OFF_C2 + NMEL
OFF_ONES = OFF_EPS + 1
SM_LEN = OFF_ONES + TOK

LN_EPS = 1e-5

_PROGRAM = None          # cached (nc, input names)
LAST_RESULTS = None      # BassKernelResults of the most recent run (for test.py)


def _build_program():
    _ensure_concourse()
    from concourse import bacc, tile
    import concourse.mybir as mybir

    f32 = mybir.dt.float32
    f32r = mybir.dt.float32r
    AF = mybir.ActivationFunctionType
    MUL = mybir.AluOpType.mult
    ADD = mybir.AluOpType.add

    nc = bacc.Bacc("TRN2", debug=False, target_bir_lowering=False)

    # All arrays are laid out host-side so every DMA is contiguous per
    # SBUF partition (partition dim leading in each tile block).
    xt_d = nc.dram_tensor("xt", [8, 128, 3, EXT], f32r, kind="ExternalInput")
    wbig_d = nc.dram_tensor("wbig", [DT, 128, KT, 128], f32r, kind="ExternalInput")
    cw1_d = nc.dram_tensor("cw1", [128, DT, 3, 128], f32r, kind="ExternalInput")
    cw2_d = nc.dram_tensor("cw2", [128, DT, 3, 128], f32r, kind="ExternalInput")
    wgt_d = nc.dram_tensor("wgt", [128, DT, NMEL], f32r, kind="ExternalInput")
    cb_d = nc.dram_tensor("cb", [128, 27], f32, kind="ExternalInput")
    sm_d = nc.dram_tensor("smalls", [1, SM_LEN], f32r, kind="ExternalInput")
    onec_d = nc.dram_tensor("onec", [128, 1], f32r, kind="ExternalInput")
    wu_d = nc.dram_tensor("wu", [128, 128], f32r, kind="ExternalInput")
    mel_d = nc.dram_tensor("mel", [NMEL, TOK], f32, kind="ExternalOutput")

    with tile.TileContext(nc) as tc:
        with (
            tc.tile_pool(name="consts", bufs=1) as consts,
            tc.tile_pool(name="wpool", bufs=2) as wpool,
            tc.tile_pool(name="acts", bufs=1) as acts,
            tc.tile_pool(name="stats", bufs=1) as stats,
            tc.tile_pool(name="ps_mm", bufs=4, space="PSUM") as ps_mm,
            tc.tile_pool(name="ps_st", bufs=1, space="PSUM") as ps_st,
            tc.tile_pool(name="ps_mel", bufs=3, space="PSUM") as ps_mel,
        ):
            # ---- tiny constants first (warmup deps) ----
            wu_sb = consts.tile([128, 128], f32r, name="wu_sb")
            nc.sync.dma_start(out=wu_sb, in_=wu_d[:])
            sm_sb = consts.tile([1, SM_LEN], f32r, name="sm_sb")
            nc.sync.dma_start(out=sm_sb, in_=sm_d[:])
            cb_sb = consts.tile([128, 27], f32, name="cb_sb")
            nc.sync.dma_start(out=cb_sb, in_=cb_d[:])
            ones_col = consts.tile([128, 1], f32r, name="ones_col")
            nc.sync.dma_start(out=ones_col, in_=onec_d[:])
            ones_row = sm_sb[0:1, OFF_ONES : OFF_ONES + TOK]

            # ---- PE warmup: ~40 dummy matmuls while input DMAs stream ----
            ps_wu = ps_mm.tile([128, 128], f32, name="ps_wu", tag="mm")
            for i in range(40):
                nc.tensor.matmul(
                    ps_wu, lhsT=wu_sb, rhs=wu_sb,
                    start=(i == 0), stop=(i == 39),
                )

            # ---- streaming DMAs: W d-slices interleaved with token chunks --
            wb_tiles = []
            xg = []

            def load_wb(d):
                t = wpool.tile([128, KT, 128], f32r, name=f"wb{d}", tag="wb")
                nc.sync.dma_start(out=t, in_=wbig_d[d])
                wb_tiles.append(t)

            def load_xg(j):
                t = consts.tile([128, 3, EXT], f32r, name=f"xg{j}", tag=f"xg{j}")
                nc.sync.dma_start(out=t, in_=xt_d[j])
                xg.append(t)

            load_wb(0)
            load_xg(0)
            load_wb(1)
            for j in range(1, 8):
                load_xg(j)

            cw1_sb = consts.tile([128, DT, 3, 128], f32r, name="cw1_sb")
            nc.sync.dma_start(out=cw1_sb, in_=cw1_d[:])
            cw2_sb = consts.tile([128, DT, 3, 128], f32r, name="cw2_sb")
            nc.sync.dma_start(out=cw2_sb, in_=cw2_d[:])
            wgt_sb = consts.tile([128, DT, NMEL], f32r, name="wgt_sb")
            nc.sync.dma_start(out=wgt_sb, in_=wgt_d[:])

            def xk(k):
                return xg[k // 3][:, k % 3, :]

            fused = [None] * DT
            g = [None] * DT
            h2 = [None] * DT
            h2sq = [None] * DT

            def gemm(d):
                if d + 2 < DT:
                    load_wb(d + 2)
                fu = acts.tile([128, EXT], f32r, name=f"fu{d}", tag=f"fu{d}")
                fused[d] = fu
                wb = wb_tiles[d]
                for h in range(2):
                    ps = ps_mm.tile([128, HB], f32, name=f"psA{d}_{h}", tag="mm")
                    sl = slice(h * HB, (h + 1) * HB)
                    for k in range(KT):
                        nc.tensor.matmul(
                            ps, lhsT=wb[:, k, :], rhs=xk(k)[:, sl],
                            start=(k == 0), stop=False,
                        )
                    nc.tensor.matmul(
                        ps,
                        lhsT=sm_sb[0:1, OFF_BB + d * 128 : OFF_BB + (d + 1) * 128],
                        rhs=sm_sb[0:1, OFF_VM + h * HB : OFF_VM + (h + 1) * HB],
                        start=False, stop=True,
                    )
                    nc.scalar.copy(out=fu[:, sl], in_=ps)

            B_STARTS = (0, 258)
            B_WIDTHS = (258, 256)

            def conv1(d):
                gd = acts.tile([128, G1E], f32r, name=f"g{d}", tag=f"g{d}")
                g[d] = gd
                for h in range(2):
                    st, w = B_STARTS[h], B_WIDTHS[h]
                    ps = ps_mm.tile([128, w], f32, name=f"psB{d}_{h}", tag="mm")
                    for tap in range(3):
                        nc.tensor.matmul(
                            ps, lhsT=cw1_sb[:, d, tap, :],
                            rhs=fused[d][:, st + tap : st + tap + w],
                            start=(tap == 0), stop=(tap == 2),
                        )
                    # exact gelu: g = (conv1+b1) * (0.5 + 0.5*erf((conv1+b1)/sqrt2))
                    e = acts.tile([128, 258], f32, name=f"e{d}_{h}", tag="e", bufs=3)
                    nc.scalar.activation(
                        out=e[:, :w], in_=ps, func=AF.Erf,
                        bias=cb_sb[:, d : d + 1], scale=0.7071067811865476,
                    )
                    h1b = acts.tile([128, 258], f32, name=f"h1b{d}_{h}", tag="h1b",
                                    bufs=3)
                    nc.vector.tensor_scalar_add(
                        h1b[:, :w], ps, cb_sb[:, 19 + d : 20 + d]
                    )
                    tt = acts.tile([128, 258], f32, name=f"tt{d}_{h}", tag="tt",
                                   bufs=3)
                    nc.vector.tensor_scalar(tt[:, :w], e[:, :w], 0.5, 0.5,
                                            op0=MUL, op1=ADD)
                    nc.vector.tensor_mul(gd[:, st : st + w], tt[:, :w], h1b[:, :w])
                # zero conv2 halo columns at sequence edges (data-driven)
                nc.vector.tensor_mul(gd[:, 0:1], gd[:, 0:1], cb_sb[:, 16:17])
                nc.vector.tensor_mul(
                    gd[:, G1E - 1 : G1E], gd[:, G1E - 1 : G1E], cb_sb[:, 17:18]
                )

            def conv2(d):
                h2d = acts.tile([128, TOK], f32r, name=f"h2{d}", tag=f"h2{d}")
                h2sqd = acts.tile([128, TOK], f32r, name=f"h2sq{d}", tag=f"h2sq{d}")
                h2[d] = h2d
                h2sq[d] = h2sqd
                for h in range(2):
                    ps = ps_mm.tile([128, H2], f32, name=f"psC{d}_{h}", tag="mm")
                    for tap in range(3):
                        nc.tensor.matmul(
                            ps, lhsT=cw2_sb[:, d, tap, :],
                            rhs=g[d][:, h * H2 + tap : h * H2 + tap + H2],
                            start=(tap == 0), stop=(tap == 2),
                        )
                    sl = slice(h * H2, (h + 1) * H2)
                    nc.scalar.add(out=h2d[:, sl], in_=ps,
                                  add=cb_sb[:, 8 + d : 9 + d])
                    nc.scalar.activation(
                        out=h2sqd[:, sl], in_=ps, func=AF.Square,
                        bias=cb_sb[:, 8 + d : 9 + d], scale=1.0,
                    )

            # software-pipelined emission: PE stays dense, ACT/DVE trail
            for d in range(DT):
                gemm(d)
                if d >= 1:
                    conv1(d - 1)
                if d >= 3:
                    conv2(d - 3)
            conv1(DT - 1)
            for d in range(DT - 3, DT):
                conv2(d)

            # ---- channel sums via ones-matmuls (shared psum slot) ----
            ps_mu = ps_st.tile([1, TOK], f32, name="ps_mu", tag="st")
            for d in range(DT):
                nc.tensor.matmul(
                    ps_mu, lhsT=ones_col, rhs=h2[d][:],
                    start=(d == 0), stop=(d == DT - 1),
                )
            mean = stats.tile([1, TOK], f32, name="mean")
            nc.vector.tensor_scalar_mul(mean, ps_mu, 1.0 / D)
            ps_sq = ps_st.tile([1, TOK], f32, name="ps_sq", tag="st")
            for d in range(DT):
                nc.tensor.matmul(
                    ps_sq, lhsT=ones_col, rhs=h2sq[d][:],
                    start=(d == 0), stop=(d == DT - 1),
                )

            # ---- stats on [1, TOK] ----
            msq = stats.tile([1, TOK], f32, name="msq")
            nc.scalar.activation(msq, mean, AF.Square)
            varr = stats.tile([1, TOK], f32, name="varr")
            nc.vector.tensor_scalar_mul(varr, ps_sq, 1.0 / D)
            var = stats.tile([1, TOK], f32, name="var")
            nc.vector.tensor_sub(var, varr, msq)
            sqv = stats.tile([1, TOK], f32, name="sqv")
            nc.scalar.activation(sqv, var, AF.Sqrt,
                                 bias=cb_sb[0:1, 18:19], scale=1.0)
            rstd32 = stats.tile([1, TOK], f32, name="rstd32")
            nc.vector.reciprocal(rstd32, sqv)
            rstd = stats.tile([1, TOK], f32r, name="rstd")
            nc.vector.tensor_copy(rstd, rstd32)
            negu = stats.tile([1, TOK], f32r, name="negu")
            nc.vector.scalar_tensor_tensor(
                negu, in0=mean, scalar=-1.0, in1=rstd32, op0=MUL, op1=MUL,
            )

            # ---- mel GEMM + rank-1 corrections ----
            ps_m = ps_mel.tile([NMEL, TOK], f32, name="ps_m", tag="mel")
            for d in range(DT):
                nc.tensor.matmul(
                    ps_m, lhsT=wgt_sb[:, d, :], rhs=h2[d][:],
                    start=(d == 0), stop=(d == DT - 1),
                )
            ps_r = ps_mel.tile([NMEL, TOK], f32, name="ps_r", tag="mel")
            nc.tensor.matmul(
                ps_r, lhsT=sm_sb[0:1, OFF_C1 : OFF_C1 + NMEL],
                rhs=negu[0:1, :], start=True, stop=False,
            )
            nc.tensor.matmul(
                ps_r, lhsT=sm_sb[0:1, OFF_C2 : OFF_C2 + NMEL],
                rhs=ones_row, start=False, stop=True,
            )
            ps_s = ps_mel.tile([NMEL, TOK], f32, name="ps_s", tag="mel")
            nc.tensor.matmul(
                ps_s, lhsT=sm_sb[0:1, OFF_ONES : OFF_ONES + NMEL],
                rhs=rstd[0:1, :], start=True, stop=True,
            )
            s_sb = stats.tile([NMEL, TOK], f32, name="s_sb")
            nc.vector.tensor_copy(s_sb, ps_s)
            out_sb = stats.tile([NMEL, TOK], f32, name="out_sb")
            nc.vector.tensor_mul(out_sb, ps_m, s_sb)
            nc.vector.tensor_add(out_sb, out_sb, ps_r)
            nc.sync.dma_start(out=mel_d[:], in_=out_sb)

    nc.compile()
    return nc


def _sigmoid64(x):
    return 1.0 / (1.0 + np.exp(-x.astype(np.float64)))


def host_prep(inputs):
    """Fold all data-independent computation; build per-core device inputs.

    Returns (shared, per_core) where shared is a dict of replicated arrays
    and per_core is a list of 8 dicts with the core-specific arrays.
    """
    f32 = np.float32
    W_in = np.asarray(inputs["W_in"], dtype=np.float64)
    Wd = np.asarray(inputs["Wd"], dtype=np.float64)
    bd = np.asarray(inputs["bd"], dtype=np.float64)
    P = np.asarray(inputs["P"], dtype=np.float64)
    smask = np.asarray(inputs["spectral_mask"], dtype=np.float64)
    b_in = np.asarray(inputs["b_in"], dtype=np.float64)

    w_proj = W_in @ Wd.T + bd[None, :]
    w_prime = P.T @ w_proj @ P
    masked_w = w_prime * _sigmoid64(smask)
    A = P @ masked_w.T @ P.T
    W_big = np.ascontiguousarray((W_in.T @ A), dtype=f32)      # [L, D]
    b_big = (b_in @ A).astype(f32)                             # [D]

    # [d, kp, k, dc] (partition-major for contiguous DMA)
    wbig_t = np.ascontiguousarray(
        W_big.reshape(KT, 128, DT, 128).transpose(2, 1, 0, 3)
    )

    def blockdiag(w):
        w = np.asarray(w, dtype=f32)  # [C, GS, 3]
        out = np.zeros((DT, 3, 128, 128), dtype=f32)
        for d in range(DT):
            for co in range(128):
                c = d * 128 + co
                blk = co // GS
                # out[d, tap, blk*GS + i, co] = w[c, i, tap]
                out[d, :, blk * GS : (blk + 1) * GS, co] = w[c].T
        return out

    cw1_t = np.ascontiguousarray(blockdiag(inputs["conv1_w"]).transpose(2, 0, 1, 3))
    cw2_t = np.ascontiguousarray(blockdiag(inputs["conv2_w"]).transpose(2, 0, 1, 3))

    Wmel = np.asarray(inputs["Wmel"], dtype=np.float64)
    ln_g = np.asarray(inputs["ln_g"], dtype=np.float64)
    ln_b = np.asarray(inputs["ln_b"], dtype=np.float64)
    bmel = np.asarray(inputs["bmel"], dtype=np.float64)
    Wg = (Wmel * ln_g[None, :]).astype(f32)                    # [NMEL, D]
    wgt_t = np.ascontiguousarray(
        Wg.T.reshape(DT, 128, NMEL).transpose(1, 0, 2)
    )  # [kp, d, m]
    c1 = (Wmel @ ln_g).astype(f32)
    c2 = (Wmel @ ln_b + bmel).astype(f32)

    cb_base = np.zeros((128, 27), dtype=f32)
    cb_base[:, 18] = LN_EPS
    b1_cols = np.asarray(inputs["conv1_b"], dtype=f32).reshape(DT, 128).T
    cb_base[:, 0:8] = b1_cols * np.float32(0.7071067811865476)  # pre-scaled for Erf
    cb_base[:, 8:16] = np.asarray(inputs["conv2_b"], dtype=f32).reshape(DT, 128).T
    cb_base[:, 19:27] = b1_cols

    llama = np.asarray(inputs["llama_embeddings"], dtype=f32).reshape(B * T, L)

    shared = dict(wbig=wbig_t, cw1=cw1_t, cw2=cw2_t, wgt=wgt_t,
                  onec=np.ones((128, 1), dtype=f32),
                  wu=np.zeros((128, 128), dtype=f32))
    per_core = []
    for c in range(NCORES):
        b, h = divmod(c, 2)
        start = b * T + h * TOK
        ext_idx = np.arange(start - 2, start + TOK + 2)
        valid = (ext_idx >= b * T) & (ext_idx < (b + 1) * T)
        xext = np.zeros((EXT, L), dtype=f32)
        xext[valid] = llama[ext_idx[valid]]
        xt = np.ascontiguousarray(
            xext.T.reshape(8, 3, 128, EXT).transpose(0, 2, 1, 3)
        )  # [j, p, kk, t]

        sm = np.zeros((1, SM_LEN), dtype=f32)
        sm[0, OFF_BB : OFF_BB + D] = b_big
        sm[0, OFF_VM : OFF_VM + EXT] = valid.astype(f32)
        sm[0, OFF_C1 : OFF_C1 + NMEL] = c1
        sm[0, OFF_C2 : OFF_C2 + NMEL] = c2
        sm[0, OFF_EPS] = LN_EPS
        sm[0, OFF_ONES : OFF_ONES + TOK] = 1.0

        cb = cb_base.copy()
        # g halo validity: col 16 -> token -1, col 17 -> token TOK
        cb[:, 16] = 1.0 if h == 1 else 0.0
        cb[:, 17] = 1.0 if h == 0 else 0.0

        per_core.append(dict(xt=xt, smalls=sm, cb=cb))
    return shared, per_core


def kernel(**inputs):
    global _PROGRAM, LAST_RESULTS
    _ensure_concourse()
    from concourse import bass_utils

    if _PROGRAM is None:
        _PROGRAM = _build_program()
    nc = _PROGRAM

    shared, per_core = host_prep(inputs)
    in_maps = [{**shared, **pc} for pc in per_core]

    res = bass_utils.run_bass_kernel_spmd(nc, in_maps, core_ids=list(range(NCORES)))
    LAST_RESULTS = res

    out = np.zeros((B, NMEL, T), dtype=np.float32)
    for c in range(NCORES):
        b, h = divmod(c, 2)
        out[b, :, h * TOK : (h + 1) * TOK] = res.results[c]["mel"]
    return out
